# revision 20
# baseline (speedup 1.0000x reference)
"""Memristive fully-connected layer on 8 Trainium2 NeuronCores.

Math: the reference interleaves pos/neg conductance columns, matmuls, and
takes the differential pair. Both columns of a pair see the same affine map
g = k_cond * w + G_OFF and the same voltages v = K_V * [x, 1], so in the
readout y = (I_pos - I_neg) / (K_V * k_cond) both G_OFF and k_cond cancel
exactly:

    y = x @ w_pos - x @ w_neg + (b_pos - b_neg)

Sharding: tensor-parallel over the 1024 output columns (128 per core).

v2 design (vs the 8081ns v1), 6210ns modeled:
  - fp16 upload across THREE DGE queues (SP + Activation HWDGE, Pool
    SWDGE), halving DMA bytes. The host casts w_pos and w_neg to fp16
    SEPARATELY and bakes a sign flip into the w_neg/b_neg halves; the
    differential then happens inside f32 PSUM accumulation (pos and
    sign-folded neg matmuls target the SAME psum columns), so the fp16
    rounding never cancels catastrophically (rel err ~3e-3 vs the 2e-2
    gate) and no subtract instruction is needed at all.
  - fp16 matmul retires 1 row/cycle, so each K-chunk costs two half-width
    matmuls at the same total PE time as one wide one. The cost model's
    PE p-state ramp keys on ABSOLUTE kernel time (full clock only after
    t>3us), so no warm-up fillers are needed and early matmuls run at
    1.2GHz — arrival-limited, not warmup-limited.
  - the readout (PSUM -> SBUF copy; DMA cannot read PSUM, and only one
    PSUM operand is allowed per instruction) is split into two psum
    groups: a 106-col group that stops ~170ns before PE-end and copies
    on DVE while PE finishes, and a 22-col sliver copied on Act (whose
    activation table is pre-warmed by a dummy copy on the idle Act
    queue). Each copy waits one PE tick; the two y DMAs ride the SP and
    Act queues in parallel, and Act's same-engine copy->DMA pair needs
    no semaphore hop.
  - bias pair rows ride in the xtb tile's tail columns (no separate
    500ns-floor DMA), covered by gate B's wait.

The walrus one-wait-per-instruction discipline is kept from v1:
  - every tile has its own slot; each DMA queue is used <= 2 deep;
  - gate matmuls make PE observe the xta/xtb DMA semaphores, so chunk
    matmuls carry only their weight tile's wait;
  - Tile's final drain is pruned to the last y DMA's semaphore; the
    kernel tail's EVSEM barrier is dropped entirely (NRT completion =
    all engine streams ended; each engine's hardware dge_drain quiesces
    its own in-flight DMAs) and the sem clear moves to the preamble.
"""

import numpy as np

import concourse.bass as bass
import concourse.mybir as mybir
import concourse.tile as tile
from concourse.bass_utils import run_bass_kernel_spmd

B, NIN, NOUT = 128, 1024, 1024
NCORES = 8
NS = NOUT // NCORES  # output columns per core
KC = NIN // 128      # contraction chunks of 128
FP32 = mybir.dt.float32
FP16 = mybir.dt.float16

_PROGRAM = None


def _prune_drain_waits(nc):
    """This walrus accepts at most ONE sync wait per instruction (any
    struct), but Tile's final drain carries one wait per semaphore. In this
    kernel every semaphore's final tick happens-before the output DMA's
    completion (inputs -> compute -> sub -> y DMA form one chain), so the
    drain only needs the y DMA's completion semaphore. Keep exactly that
    wait and drop the rest."""
    y_sems = set()
    for f in nc.m.functions:
        for blk in f.blocks:
            for inst in blk.instructions:
                if type(inst).__name__ != "InstDMACopy":
                    continue
                si = inst.sync_info
                y_sems = {u.id for u in (si.on_update if si else [])}
    for f in nc.m.functions:
        for blk in f.blocks:
            for inst in blk.instructions:
                if type(inst).__name__ != "InstDrain":
                    continue
                si = inst.sync_info
                waits = list(si.on_wait) if si and si.on_wait else []
                if len(waits) <= 1:
                    continue
                keep = [w for w in waits if w.id in y_sems]
                assert keep, f"drain lost its y wait: {[w.ant_name for w in waits]}"
                inst.sync_info = mybir.SyncInfo(
                    on_wait=keep, on_update=list(si.on_update) if si else []
                )
    # safety: nothing else may exceed one wait
    for f in nc.m.functions:
        for blk in f.blocks:
            for inst in blk.instructions:
                si = getattr(inst, "sync_info", None)
                nw = len(si.on_wait) if si and si.on_wait else 0
                assert nw <= 1, (
                    f"{inst.name} ({type(inst).__name__}) has {nw} waits"
                )
    return nc


def _strip_tail(nc):
    """Tile's kernel tail is [drain][all-engine barrier][sem clear][barrier]
    (~2us). The pruned drain already guarantees the output DMA landed, and
    the EVSEM barrier sems self-reset, so the only state the tail must
    restore is the Tile semaphore range — move that single sem-clear ISA op
    into the preamble (before the first barrier) and drop everything after
    the drain. Each execution then starts from zeroed semaphores."""
    func = nc.m.functions[0]
    eb = [b for b in func.blocks if b.name.endswith("_end")][-1]
    insts = list(eb.instructions)
    isa_idx = next(
        i for i, inst in enumerate(insts) if type(inst).__name__ == "InstISA"
    )
    isa = insts[isa_idx]
    # keep the pruned drain and the per-engine dge_drains (each engine
    # quiesces its own DMA queues before its stream ends — on hardware the
    # drain op itself guarantees that engine's in-flight DMAs completed),
    # but drop the end-of-kernel EVSEM barrier: NRT only signals completion
    # once every engine stream has ended, so aligning the streams buys
    # nothing, and the next execution's preamble barrier re-syncs engines
    # after the semaphore clear. Barrier drains lose their release-sem
    # waits (the release EVSEMs are gone).
    kept = []
    for inst in insts[:isa_idx]:
        t = type(inst).__name__
        if t == "InstEventSemaphore":
            continue
        if t == "InstDrain":
            si = inst.sync_info
            waits = list(si.on_wait) if si and si.on_wait else []
            if any("barrier" in w.ant_name for w in waits):
                inst.sync_info = mybir.SyncInfo(on_wait=[], on_update=[])
        kept.append(inst)
    eb.instructions = kept

    mb = func.blocks[0]
    mi = list(mb.instructions)
    fi = next(
        i for i, inst in enumerate(mi) if type(inst).__name__ == "InstDrain"
    )
    mb.instructions = mi[:fi] + [isa] + mi[fi:]
    return nc


def _build(split=True):
    nc = bass.Bass()
    xta = nc.declare_dram_parameter("xta", [128, 4 * B], FP16, isOutput=False)
    xtb = nc.declare_dram_parameter("xtb", [128, 4 * B + 2 * NS], FP16, isOutput=False)
    wa = nc.declare_dram_parameter("wa", [128, 4 * NS], FP16, isOutput=False)
    wb = nc.declare_dram_parameter("wb", [128, 4 * NS], FP16, isOutput=False)
    wc = nc.declare_dram_parameter("wc", [128, 8 * NS], FP16, isOutput=False)
    y = nc.declare_dram_parameter("y", [B, NS], FP32, isOutput=True)

    with tile.TileContext(nc) as tc:
        with (
            tc.tile_pool(name="xpool", bufs=1) as xpool,
            tc.tile_pool(name="wpool", bufs=1) as wpool,
            tc.tile_pool(name="misc", bufs=1) as misc,
            tc.tile_pool(name="opool", bufs=1) as opool,
            tc.tile_pool(name="opool2", bufs=1) as opool2,
            tc.tile_pool(name="psum", bufs=1, space="PSUM") as psum_pool,
        ):
            # DMA schedule: first-needed tensors take each queue's first
            # slot (fixed DGE latency then overlaps across queues).
            #   SP (sync)  : xta | xtb(+bias row) | y
            #   Act (scalar): wa | wb
            #   Pool (gpsimd SWDGE): wc
            xta_t = xpool.tile([128, 4 * B], FP16, tag="xta")
            nc.sync.dma_start(xta_t[:], xta[:])
            wa_t = wpool.tile([128, 4 * NS], FP16, tag="wa")
            nc.scalar.dma_start(wa_t[:], wa[:])
            wc_t = wpool.tile([128, 8 * NS], FP16, tag="wc")
            nc.gpsimd.dma_start(wc_t[:], wc[:])
            xtb_t = xpool.tile([128, 4 * B + 2 * NS], FP16, tag="xtb")
            nc.sync.dma_start(xtb_t[:], xtb[:])
            wb_t = wpool.tile([128, 4 * NS], FP16, tag="wb")
            nc.scalar.dma_start(wb_t[:], wb[:])

            # all-ones fp16 row for the bias matmul's stationary operand
            # (the cost model's PE p-state ramp keys on absolute kernel
            # time, so no warm-up fillers are needed — verified empirically)
            flt_t = misc.tile([1, B], FP16, name="flt")
            nc.vector.memset(flt_t[:], 1.0)

            # dummy activation on the idle Act queue: pays the 1.3us
            # activation-table load early so the readout-split copy on Act
            # later costs only the op itself
            dummy_t = misc.tile([1, 1], FP16, name="actwarm")
            nc.scalar.copy(dummy_t[:], flt_t[0:1, 0:1])

            # pos and neg currents accumulate into the SAME psum columns
            # (the host bakes a sign flip into the w_neg/b_neg halves), so
            # the differential subtract happens inside PSUM accumulation
            # hardware and the readout is a plain copy. Two column groups:
            # the wide one stops early so its copy/DMA overlap the sliver's
            # last matmuls, and each copy carries one PE-tick wait.
            NSV = 106
            ps = psum_pool.tile([B, NSV], FP32)
            ps_r = psum_pool.tile([B, NS - NSV], FP32, name="psr")

            def xt_chunk(c):
                t = xta_t if c < 4 else xtb_t
                lo = (c % 4) * B
                return t[:, lo : lo + B]

            # gate A: PE observes xta's DMA lane; chunks 0-3 then carry only
            # their weight tile's wait
            gate_ps = psum_pool.tile([B, 1], FP32)
            nc.tensor.matmul(
                gate_ps[:], xta_t[:, 0:B], xta_t[:, 0:1], start=True, stop=True
            )
            w_src = {0: wa_t, 1: wa_t, 2: wb_t, 3: wb_t,
                     4: wc_t, 5: wc_t, 6: wc_t, 7: wc_t}
            w_off = {0: 0, 1: 2 * NS, 2: 0, 3: 2 * NS,
                     4: 0, 5: 2 * NS, 6: 4 * NS, 7: 6 * NS}

            def emit_chunk(g, ps_t, lo, hi, start, stop):
                # pos then (sign-folded) neg half into the same psum cols
                for h in range(2):
                    nc.tensor.matmul(
                        ps_t[:],
                        xt_chunk(g),
                        w_src[g][:, w_off[g] + h * NS + lo : w_off[g] + h * NS + hi],
                        start=start and h == 0,
                        stop=stop and h == 1,
                    )

            def emit_bias(ps_t, lo, hi, stop):
                # bias pair rows: ones[1,B] x b_pos / -b_neg; the first
                # waits only the DVE memset semaphore (xtb via gate B)
                for h in range(2):
                    nc.tensor.matmul(
                        ps_t[:],
                        flt_t[:],
                        xtb_t[0:1, 4 * B + h * NS + lo : 4 * B + h * NS + hi],
                        start=False,
                        stop=stop and h == 1,
                    )

            for g in range(4):
                emit_chunk(g, ps, 0, NSV, start=(g == 0), stop=False)
            # gate B: PE observes xtb's DMA lane (covers chunks 4-7's
            # stationary operands AND the bias row in its tail columns)
            gate_ps2 = psum_pool.tile([B, 1], FP32)
            nc.tensor.matmul(
                gate_ps2[:], xtb_t[:, 0:B], xtb_t[:, 0:1], start=True, stop=True
            )
            for g in range(4, KC):
                emit_chunk(g, ps, 0, NSV, start=False, stop=False)
            emit_bias(ps, 0, NSV, stop=True)
            # sliver group: runs after the wide group so the wide copy and
            # its y DMA overlap these last matmuls
            for g in range(KC):
                emit_chunk(g, ps_r, NSV, NS, start=(g == 0), stop=False)
            emit_bias(ps_r, NSV, NS, stop=True)

            # y already differential in PSUM; the readout is a plain copy
            # (hardware allows only one PSUM operand per instruction, and
            # GPSIMD cannot access PSUM at all). Wide group on DVE as soon
            # as its group stops; sliver on (table-warmed) Act at PE end;
            # y DMAs ride the SP and Act queues in parallel.
            out0 = opool.tile([B, NSV], FP32, name="out0")
            out1 = opool2.tile([B, NS - NSV], FP32, name="out1")
            nc.vector.tensor_copy(out0[:], ps[:])
            nc.scalar.copy(out1[:], ps_r[:])
            nc.scalar.dma_start(y[:, NSV:NS], out1[:])
            nc.sync.dma_start(y[:, 0:NSV], out0[:])
    return _strip_tail(_prune_drain_waits(nc)) if split else nc


def _program():
    global _PROGRAM
    if _PROGRAM is None:
        _PROGRAM = _build()
    return _PROGRAM


def _in_maps(x, w_pos, w_neg, b_pos, b_neg):
    x = np.asarray(x, dtype=np.float32)
    w_pos = np.asarray(w_pos, dtype=np.float32)
    w_neg = np.asarray(w_neg, dtype=np.float32)
    b_pos = np.asarray(b_pos, dtype=np.float32)
    b_neg = np.asarray(b_neg, dtype=np.float32)
    # x^T in K-chunk-major tile layout: chunk c cols hold x[:, c*128+p]^T
    xt = np.ascontiguousarray(x.T.astype(np.float16))  # [NIN, B]
    xt_r = xt.reshape(KC, 128, B)
    xta = np.ascontiguousarray(
        np.concatenate([xt_r[c] for c in range(4)], axis=1)
    )
    wp16 = w_pos.astype(np.float16)
    wn16 = w_neg.astype(np.float16)
    maps = []
    for j in range(NCORES):
        sl = slice(j * NS, (j + 1) * NS)
        xtb = np.zeros((128, 4 * B + 2 * NS), dtype=np.float16)
        xtb[:, : 4 * B] = np.concatenate(
            [xt_r[c] for c in range(4, KC)], axis=1
        )
        xtb[0, 4 * B : 4 * B + NS] = b_pos[sl].astype(np.float16)
        xtb[0, 4 * B + NS : 4 * B + 2 * NS] = -b_neg[sl].astype(np.float16)

        def wtile(chunks):
            out = np.empty((128, len(chunks) * 2 * NS), dtype=np.float16)
            for i, c in enumerate(chunks):
                rows = slice(c * 128, (c + 1) * 128)
                out[:, i * 2 * NS : i * 2 * NS + NS] = wp16[rows, sl]
                out[:, i * 2 * NS + NS : (i + 1) * 2 * NS] = -wn16[rows, sl]
            return out

        maps.append(
            {
                "xta": xta,
                "xtb": xtb,
                "wa": wtile([0, 1]),
                "wb": wtile([2, 3]),
                "wc": wtile([4, 5, 6, 7]),
            }
        )
    return maps


def kernel(x, w_pos, w_neg, b_pos, b_neg):
    maps = _in_maps(x, w_pos, w_neg, b_pos, b_neg)
    res = run_bass_kernel_spmd(_program(), maps, list(range(NCORES))).results
    return np.concatenate([res[j]["y"] for j in range(NCORES)], axis=1)
